# revision 7
# baseline (speedup 1.0000x reference)
"""Correlation cost-volume kernel (max_displacement=4) for 8 Trainium2 cores.

Problem: in1, in2: [B=8, C=256, H=128, W=128] f32.
out[b, dy*9+dx, h, w] = sum_c in1[b,c,h,w] * pad(in2)[b, c, h+dy, w+dx]
(pad = 4 zeros on each spatial side), output [8, 81, 128, 128] f32.

Strategy (data-parallel, one batch sample per core):
  2D-tiled gram.  Each matmul tile covers an 8h x 16w block of in1 pixels
  (M = 128 PSUM partitions = pixels) against its 16 x 24 halo region of
  padded in2 (N = 384 columns, contracting C = 256 as two K=128 tiles
  accumulated in PSUM).  Every (pixel, displacement) pair the band needs is
  a (partition, column) entry of that [128 x 384] tile-gram, so the device
  streams 2x384 columns per 128 pixels -- 3.2x less TensorE work and 3.2x
  less output DMA than the per-row full-gram formulation.  The band entry
  for pixel (mh,mw) sits at column (mh+dy)*24 + (mw+dx): a per-partition
  (sheared) offset that no engine or DMA access pattern can express, so the
  device emits the full tile-grams (bf16, 4 tiles batched per DMA) and the
  host slices the 81-entry band per pixel with numpy stride tricks.
"""

import ml_dtypes
import numpy as np

import concourse.bass as bass
import concourse.bacc as bacc
import concourse.mybir as mybir
from concourse.bass_utils import run_bass_kernel_spmd
from concourse.tile import TileContext

B, C, H, W = 8, 256, 128, 128
D = 4
ND = 2 * D + 1  # 9 displacements per axis
HP = H + 2 * D  # 136 padded rows
WP = W + 2 * D  # 136 padded cols
KT = C // 128  # 2 contraction tiles
TH, TW = 8, 16  # matmul tile = 8h x 16w pixels (128 = one PSUM partition dim)
NHT, NWT = H // TH, W // TW  # 16 x 8 tiles
RH, RW = TH + 2 * D, TW + 2 * D  # 16 x 24 halo region, N = 384
NR = RH * RW  # 384 gram columns per tile
SB = 4  # tiles batched per output DMA

OUT_DT = mybir.dt.bfloat16
_OUT_NP = ml_dtypes.bfloat16

_CACHED_NC = None


def _build_nc():
    bf16 = mybir.dt.bfloat16

    nc = bacc.Bacc()
    # in1 as [c][ht][kt][wt][m=mh*16+mw]; in2 zero-padded as [c][kt][hp][wp]
    in1_t = nc.declare_dram_parameter("in1_t", [128, NHT, KT, NWT, 128], bf16, isOutput=False)
    in2_p = nc.declare_dram_parameter("in2_p", [128, KT, HP, WP], bf16, isOutput=False)
    # tile-grams: [ht][j][m][q][n] with wt = 4j+q; partition m contiguous run
    out_g = nc.declare_dram_parameter(
        "out_g", [NHT, NWT // SB, 128, SB, NR], OUT_DT, isOutput=True
    )

    with TileContext(nc) as tc:
        with (
            tc.tile_pool(name="bpool", bufs=1) as bpool,
            tc.tile_pool(name="apool", bufs=1) as apool,
            tc.tile_pool(name="spool", bufs=3) as spool,
            tc.tile_pool(name="psum", bufs=6, space="PSUM") as ppool,
        ):
            # whole padded in2 sample resident in SBUF (74KB/partition),
            # loaded in 16-row chunks so early tiles can start before the
            # full 9.5MB lands (subtile deps give matmuls per-chunk waits)
            b_s = bpool.tile([128, KT, HP, WP], bf16)
            # whole in1 sample resident (64KB/partition), 2-ht chunks (1MB DMAs)
            a_s = apool.tile([128, NHT, KT, NWT, 128], bf16)

            def load_b(k):  # 16-row chunk k of padded in2
                r0 = 16 * k
                nr = min(16, HP - r0)
                nc.sync.dma_start(
                    out=b_s[:, :, r0 : r0 + nr], in_=in2_p[:, :, r0 : r0 + nr]
                )

            def load_a(t):  # 2-ht chunk t of in1
                nc.sync.dma_start(
                    out=a_s[:, 2 * t : 2 * t + 2], in_=in1_t[:, 2 * t : 2 * t + 2]
                )

            # all input DMAs issued up front (before any matmul reads, so no
            # conservative WAR hazards throttle the stream); output DMAs go on
            # the scalar HWDGE ring, so they drain concurrently instead of
            # queueing behind the whole input stream
            for k in range(9):
                load_b(k)
                if k < 8:
                    load_a(k)

            st = None
            for ht in range(NHT):
                h0 = TH * ht
                for wt in range(NWT):
                    w0 = TW * wt
                    idx = ht * NWT + wt
                    ps = ppool.tile([128, NR], mybir.dt.float32, name="ps", tag="ps")
                    for kt in range(KT):
                        nc.tensor.matmul(
                            ps,
                            a_s[:, ht, kt, wt, :],
                            b_s[:, kt, h0 : h0 + RH, w0 : w0 + RW],
                            start=(kt == 0),
                            stop=(kt == KT - 1),
                        )
                    if idx % SB == 0:
                        st = spool.tile([128, SB, NR], OUT_DT)
                    if idx % 2 == 0:
                        nc.vector.tensor_copy(st[:, idx % SB, :], ps)
                    else:
                        nc.scalar.copy(st[:, idx % SB, :], ps)
                    if idx % SB == SB - 1:
                        # scalar-engine HWDGE queue: keeps output DMAs off the
                        # sync queue so they don't head-of-line block input
                        # prefetches (and vice versa)
                        nc.scalar.dma_start(
                            out=out_g[ht, (wt - SB + 1) // SB],
                            in_=st,
                        )

    # bacc passes (move_matmul_waits_to_ldweights / generate_event_semaphores)
    # enforce the 1-wait-per-instruction HW constraint before serialization.
    nc.compile()
    return nc


def _get_nc():
    global _CACHED_NC
    if _CACHED_NC is None:
        _CACHED_NC = _build_nc()
    return _CACHED_NC


def _make_in_maps(in1: np.ndarray, in2: np.ndarray):
    in_maps = []
    for b in range(B):
        # [C,H,W] -> [c(128), ht, kt, wt, mh*16+mw]
        a = (
            in1[b]
            .astype(ml_dtypes.bfloat16)
            .reshape(KT, 128, NHT, TH, NWT, TW)
            .transpose(1, 2, 0, 4, 3, 5)
            .reshape(128, NHT, KT, NWT, 128)
        )
        p = np.zeros((KT, 128, HP, WP), ml_dtypes.bfloat16)
        p[:, :, D : D + H, D : D + W] = in2[b].astype(ml_dtypes.bfloat16).reshape(
            KT, 128, H, W
        )
        in_maps.append(
            {
                "in1_t": np.ascontiguousarray(a),
                "in2_p": np.ascontiguousarray(p.transpose(1, 0, 2, 3)),
            }
        )
    return in_maps


_IH = np.arange(TH)[:, None]
_JW = np.arange(TW)[None, :]


def _extract_band(g: np.ndarray) -> np.ndarray:
    """[NHT, NWT//SB, 128, SB, NR] tile-grams -> [81, H, W] cost volume."""
    # -> [ht, wt, mh, mw, nh, nw]
    g2 = (
        g.reshape(NHT, NWT // SB, 128, SB, NR)
        .transpose(0, 1, 3, 2, 4)
        .reshape(NHT, NWT, TH, TW, RH, RW)
        .astype(np.float32)
    )
    # windows over (nh, nw): sw[.., a, b, dy, dx] = g2[.., a+dy, b+dx]
    sw = np.lib.stride_tricks.sliding_window_view(g2, (ND, ND), axis=(4, 5))
    band = sw[:, :, _IH, _JW, _IH, _JW]  # [ht, wt, mh, mw, dy, dx]
    # -> [dy, dx, ht, mh, wt, mw] -> [81, H, W]
    return np.ascontiguousarray(band.transpose(4, 5, 0, 2, 1, 3)).reshape(ND * ND, H, W)


def kernel(**inputs) -> np.ndarray:
    in1 = np.ascontiguousarray(np.asarray(inputs["in1"], dtype=np.float32))
    in2 = np.ascontiguousarray(np.asarray(inputs["in2"], dtype=np.float32))
    assert in1.shape == (B, C, H, W) and in2.shape == (B, C, H, W)

    nc = _get_nc()
    in_maps = _make_in_maps(in1, in2)
    res = run_bass_kernel_spmd(nc, in_maps, list(range(B)))

    outs = [_extract_band(np.asarray(res.results[b]["out_g"])) for b in range(B)]
    return np.stack(outs).astype(np.float32)


# revision 12
# speedup vs baseline: 1.1051x; 1.1051x over previous
"""Correlation cost-volume kernel (max_displacement=4) for 8 Trainium2 cores.

Problem: in1, in2: [B=8, C=256, H=128, W=128] f32.
out[b, dy*9+dx, h, w] = sum_c in1[b,c,h,w] * pad(in2)[b, c, h+dy, w+dx]
(pad = 4 zeros on each spatial side), output [8, 81, 128, 128] f32.

Strategy (data-parallel, one batch sample per core):
  2D-tiled gram.  Each matmul tile covers an 8h x 16w block of in1 pixels
  (M = 128 PSUM partitions = pixels) against its 16 x 24 halo region of
  padded in2 (N = 384 columns, contracting C = 256 as two K=128 tiles
  accumulated in PSUM).  Every (pixel, displacement) pair the band needs is
  a (partition, column) entry of that [128 x 384] tile-gram, so the device
  streams 2x384 columns per 128 pixels -- 3.2x less TensorE work and 3.2x
  less output DMA than the per-row full-gram formulation.  The band entry
  for pixel (mh,mw) sits at column (mh+dy)*24 + (mw+dx): a per-partition
  (sheared) offset that no engine or DMA access pattern can express, so the
  device emits the full tile-grams (bf16, 4 tiles batched per DMA) and the
  host slices the 81-entry band per pixel with numpy stride tricks.
"""

import ml_dtypes
import numpy as np

import concourse.bass as bass
import concourse.bacc as bacc
import concourse.mybir as mybir
from concourse.bass_utils import run_bass_kernel_spmd
from concourse.tile import TileContext

B, C, H, W = 8, 256, 128, 128
D = 4
ND = 2 * D + 1  # 9 displacements per axis
HP = H + 2 * D  # 136 padded rows
WP = W + 2 * D  # 136 padded cols
KT = C // 128  # 2 contraction tiles
TH, TW = 8, 16  # matmul tile = 8h x 16w pixels (128 = one PSUM partition dim)
NHT, NWT = H // TH, W // TW  # 16 x 8 tiles
RH, RW = TH + 2 * D, TW + 2 * D  # 16 x 24 halo region, N = 384
NR = RH * RW  # 384 gram columns per tile

OUT_DT = mybir.dt.bfloat16
_OUT_NP = ml_dtypes.bfloat16

_CACHED_NC = None


def _build_nc():
    bf16 = mybir.dt.bfloat16

    nc = bacc.Bacc()
    # in1 as [c][ht][kt][wt][m=mh*16+mw]; in2 zero-padded as [c][kt][hp][wp]
    in1_t = nc.declare_dram_parameter("in1_t", [128, NHT, KT, NWT, 128], bf16, isOutput=False)
    in2_p = nc.declare_dram_parameter("in2_p", [128, KT, HP, WP], bf16, isOutput=False)
    # tile-grams: [ht][m][wt][n]; partition m writes one contiguous 6KB run
    out_g = nc.declare_dram_parameter(
        "out_g", [NHT, 128, NWT, NR], OUT_DT, isOutput=True
    )

    with TileContext(nc) as tc:
        with (
            tc.tile_pool(name="bpool", bufs=1) as bpool,
            tc.tile_pool(name="apool", bufs=1) as apool,
            # deep output staging: ~56 tile-grams can sit in SBUF, so compute
            # never stalls on the output DMAs that the single HWDGE ring
            # serializes behind the whole input stream; after the last input
            # lands the drain runs at pure HBM-write rate, not compute rate
            tc.tile_pool(name="spool", bufs=7) as spool,
            tc.tile_pool(name="psum", bufs=6, space="PSUM") as ppool,
        ):
            # whole padded in2 sample resident in SBUF (74KB/partition),
            # loaded in 16-row chunks so early tiles can start before the
            # full 9.5MB lands (subtile deps give matmuls per-chunk waits)
            b_s = bpool.tile([128, KT, HP, WP], bf16)
            # whole in1 sample resident (64KB/partition), 2-ht chunks (1MB DMAs)
            a_s = apool.tile([128, NHT, KT, NWT, 128], bf16)

            def load_b(k):  # 16-row chunk k of padded in2
                r0 = 16 * k
                nr = min(16, HP - r0)
                nc.sync.dma_start(
                    out=b_s[:, :, r0 : r0 + nr], in_=in2_p[:, :, r0 : r0 + nr]
                )

            def load_a(t):  # 2-ht chunk t of in1
                nc.sync.dma_start(
                    out=a_s[:, 2 * t : 2 * t + 2], in_=in1_t[:, 2 * t : 2 * t + 2]
                )

            # all input DMAs issued up front (before any matmul reads, so no
            # conservative WAR hazards throttle the stream); output DMAs go on
            # the scalar HWDGE ring, so they drain concurrently instead of
            # queueing behind the whole input stream
            for k in range(9):
                load_b(k)
                if k < 8:
                    load_a(k)

            st = None
            for ht in range(NHT):
                h0 = TH * ht
                for wt in range(NWT):
                    w0 = TW * wt
                    idx = ht * NWT + wt
                    ps = ppool.tile([128, NR], mybir.dt.float32, name="ps", tag="ps")
                    for kt in range(KT):
                        nc.tensor.matmul(
                            ps,
                            a_s[:, ht, kt, wt, :],
                            b_s[:, kt, h0 : h0 + RH, w0 : w0 + RW],
                            start=(kt == 0),
                            stop=(kt == KT - 1),
                        )
                    if wt == 0:
                        st = spool.tile([128, NWT, NR], OUT_DT)
                    if idx % 2 == 0:
                        nc.vector.tensor_copy(st[:, wt, :], ps)
                    else:
                        nc.scalar.copy(st[:, wt, :], ps)
                    if wt == NWT - 1:
                        nc.sync.dma_start(out=out_g[ht], in_=st)

    # bacc passes (move_matmul_waits_to_ldweights / generate_event_semaphores)
    # enforce the 1-wait-per-instruction HW constraint before serialization.
    nc.compile()
    return nc


def _get_nc():
    global _CACHED_NC
    if _CACHED_NC is None:
        _CACHED_NC = _build_nc()
    return _CACHED_NC


def _make_in_maps(in1: np.ndarray, in2: np.ndarray):
    in_maps = []
    for b in range(B):
        # [C,H,W] -> [c(128), ht, kt, wt, mh*16+mw]
        a = (
            in1[b]
            .astype(ml_dtypes.bfloat16)
            .reshape(KT, 128, NHT, TH, NWT, TW)
            .transpose(1, 2, 0, 4, 3, 5)
            .reshape(128, NHT, KT, NWT, 128)
        )
        p = np.zeros((KT, 128, HP, WP), ml_dtypes.bfloat16)
        p[:, :, D : D + H, D : D + W] = in2[b].astype(ml_dtypes.bfloat16).reshape(
            KT, 128, H, W
        )
        in_maps.append(
            {
                "in1_t": np.ascontiguousarray(a),
                "in2_p": np.ascontiguousarray(p.transpose(1, 0, 2, 3)),
            }
        )
    return in_maps


_IH = np.arange(TH)[:, None]
_JW = np.arange(TW)[None, :]


def _extract_band(g: np.ndarray) -> np.ndarray:
    """[NHT, 128, NWT, NR] tile-grams -> [81, H, W] cost volume."""
    # -> [ht, wt, mh, mw, nh, nw]
    g2 = (
        g.reshape(NHT, 128, NWT, NR)
        .transpose(0, 2, 1, 3)
        .reshape(NHT, NWT, TH, TW, RH, RW)
        .astype(np.float32)
    )
    # windows over (nh, nw): sw[.., a, b, dy, dx] = g2[.., a+dy, b+dx]
    sw = np.lib.stride_tricks.sliding_window_view(g2, (ND, ND), axis=(4, 5))
    band = sw[:, :, _IH, _JW, _IH, _JW]  # [ht, wt, mh, mw, dy, dx]
    # -> [dy, dx, ht, mh, wt, mw] -> [81, H, W]
    return np.ascontiguousarray(band.transpose(4, 5, 0, 2, 1, 3)).reshape(ND * ND, H, W)


def kernel(**inputs) -> np.ndarray:
    in1 = np.ascontiguousarray(np.asarray(inputs["in1"], dtype=np.float32))
    in2 = np.ascontiguousarray(np.asarray(inputs["in2"], dtype=np.float32))
    assert in1.shape == (B, C, H, W) and in2.shape == (B, C, H, W)

    nc = _get_nc()
    in_maps = _make_in_maps(in1, in2)
    res = run_bass_kernel_spmd(nc, in_maps, list(range(B)))

    outs = [_extract_band(np.asarray(res.results[b]["out_g"])) for b in range(B)]
    return np.stack(outs).astype(np.float32)
